# revision 13
# baseline (speedup 1.0000x reference)
"""Differentiable-histogram Trainium2 kernel (256 triangular bins), v2.

Contract: kernel(**inputs) takes the FULL inputs from setup_inputs()
(images_batch: (8,3,256,256) f32 in [0,1]; bin_centers: (256,) f32 =
linspace(0,1,256)) and returns the FULL (8,256) f32 histogram:

    hist[b, j] = sum_i relu(1 - |255*x_bi - j|)

Strategy (pure data parallel, one image per NeuronCore, 8 cores):
  Host uploads t16 = round(x*255*256) as uint16 (fixed point 8.8, halves
  the DMA bytes; quantization error <= 1/512 in t).  On device, with
  h = t16 >> 12 (coarse bin, 16 wide) and r16 = t16 & 0xFFF (= 256*r,
  r in [0,16)):
    A side (16 slots, matmul stationary): [ones, S_0..S_14],
      S_a = 1{t16 > 4096(a+1)-0.5} = 1{h > a}  -- DVE is_gt straight off
      t16, no float prologue needed.
    C side (17 slots, matmul moving): [ones, R_0..R_15],
      R_lam = relu(r16 - 256*lam) -- one pass per column, split across
      DVE (tensor_scalar sub/max), ScalarE (activation Relu) and GPSIMD
      (tensor_scalar) to use all three elementwise engines.
  One matmul per FOLD pixel-columns accumulates A^T C into a single PSUM
  bank; host extracts the fold diagonal and decodes with first/second
  differences (tri(r-l) = clamp01(r-l+1) - clamp01(r-l), and
  relu(r-lam) = sum_{mu>=lam} clamp01(r-mu)).

Perf structure: input DMA for chunk0 issued from the ScalarE queue
(ready ~3us before sync's), dummy Relu to pull ACT_TABLE_LOAD into the
DMA window, a burst of dummy matmuls to warm the PE HAM clock gate
before the real matmul stream, no SBUF buffer reuse (no WAR stalls),
and a small final chunk to keep the exposed matmul tail short.
"""

import json as _json
from contextlib import ExitStack

import numpy as np

import concourse.bass as bass
import concourse.tile as tile
from concourse import mybir
from concourse.bass_utils import run_bass_kernel_spmd

FP32 = mybir.dt.float32
BF16 = mybir.dt.bfloat16
U16 = mybir.dt.uint16
ALU = mybir.AluOpType
ACT = mybir.ActivationFunctionType

N_CORES = 8
P, F = 128, 1536  # per-core pixels: 3*256*256 = 196608 = 128*1536
FOLD = 8
M_A = 16  # lhsT slots: ones + S_0..S_14
N_C = 17  # rhs slots: ones + R_0..R_15
MR, MC = M_A * FOLD, N_C * FOLD

CHUNKS = (768, 640, 128)
# C-column engine assignment (lam -> engine). GPSIMD takes the low lams,
# ScalarE the middle, DVE the rest.
SC_COUNT = (6, 8, 8)  # per-chunk: how many C columns ScalarE computes
N_WARM_MM = 18   # dummy matmuls at startup to warm the PE HAM clock gate
RR_MODE = "and"  # "and" | "mod" : how r16 is computed from t16
RELU_BIAS_IMM = False  # immediate bias on ScalarE Relu (else bias tiles)


def _split_multiwaits(bir_bytes: bytes) -> bytes:
    """This container's walrus rejects any instruction carrying more than
    one sem wait. Split extras onto standalone EventSemaphore instructions;
    additionally drop the exit-drain's queue waits (NRT drains rings at
    exec end anyway)."""
    bir = _json.loads(bir_bytes)
    for fn in bir["functions"]:
        for blk in fn["blocks"]:
            is_end = str(blk.get("name", "")).endswith("_end")
            out = []
            for ins in blk["instructions"]:
                si = ins.get("sync_info")
                ow = (si or {}).get("on_wait") or []
                if is_end and ins.get("opcode") == "Drain" and len(ow) > 1:
                    si["on_wait"] = []
                elif len(ow) > 1:
                    for k, w in enumerate(ow[:-1]):
                        out.append(
                            {
                                "debug": ins.get("debug", 1),
                                "engine": ins["engine"],
                                "ins": [],
                                "name": f"{ins['name']}_w{k}",
                                "opcode": "EventSemaphore",
                                "outs": [],
                                "sync_info": {"on_update": [], "on_wait": [w]},
                            }
                        )
                    si["on_wait"] = [ow[-1]]
                out.append(ins)
            blk["instructions"] = out
    return _json.dumps(bir).encode()


def _build_program(chunks=CHUNKS, sc_count=SC_COUNT,
                   n_warm=N_WARM_MM, rr_mode=RR_MODE,
                   relu_bias_imm=RELU_BIAS_IMM):
    assert sum(chunks) == F
    n_mm = sum(gc // FOLD for gc in chunks)
    sc_lams = tuple(range(max(sc_count)))

    nc = bass.Bass("TRN2", target_bir_lowering=False)

    x_dram = nc.dram_tensor("x", [P, F], U16, kind="ExternalInput")
    gacc_dram = nc.dram_tensor("gacc", [MR, 2 * MC], FP32, kind="ExternalOutput")

    with tile.TileContext(nc) as tc, ExitStack() as ctx:
        singles = ctx.enter_context(tc.tile_pool(name="singles", bufs=1))
        psum_pool = ctx.enter_context(tc.tile_pool(name="psum", bufs=1, space="PSUM"))

        # Two accumulator banks: chunks 0..n-2 into ps0, last chunk into
        # ps1, so ps0's stage copy + output DMA overlap the final chunk.
        ps0 = psum_pool.tile([MR, MC], FP32, name="ps0")
        ps1 = psum_pool.tile([MR, MC], FP32, name="ps1")
        ps_warm = psum_pool.tile([P, 512], FP32, name="ps_warm")

        # Per-chunk tiles, no reuse anywhere (no WAR deps, no self-pacing).
        xcs, rrs, a_ts, c_ts = [], [], [], []
        for ci, gc in enumerate(chunks):
            ng = gc // FOLD
            xcs.append(singles.tile([P, gc], U16, name=f"x{ci}"))
            rrs.append(singles.tile([P, gc], U16, name=f"r{ci}"))
            a_ts.append(singles.tile([P, ng, M_A, FOLD], BF16, name=f"A{ci}"))
            c_ts.append(singles.tile([P, N_C, gc], BF16, name=f"C{ci}"))
        warm = singles.tile([P, 512], BF16, name="warm")
        dummy = singles.tile([P, 1], FP32, name="dummy")
        dummy2 = singles.tile([P, 1], FP32, name="dummy2")

        bias_t = {}
        for lam in sc_lams:
            bt = singles.tile([P, 1], FP32, name=f"bias{lam}")
            nc.vector.memset(bt[:], float(-256 * lam))
            bias_t[lam] = bt

        # --- input DMAs first, spread across the three DMA-capable queues
        # in the order their preambles clear: gpsimd (~7.2us), sync (~7.6),
        # scalar (~7.3, before its ACT_TABLE_LOAD).
        # chunk0 is the critical path: split its transfer across the scalar
        # and sync queues so it lands ~1.4us sooner; x1 follows on scalar,
        # x2 on sync.  gpsimd's queue stays free for the ones memsets.
        h0 = chunks[0] // 2
        c0, c1, c2 = chunks
        nc.scalar.dma_start(xcs[0][:, 0:h0], x_dram[:, 0:h0])
        nc.sync.dma_start(xcs[0][:, h0:], x_dram[:, h0:c0])
        nc.scalar.dma_start(xcs[1][:], x_dram[:, c0 : c0 + c1])
        nc.sync.dma_start(xcs[2][:], x_dram[:, c0 + c1 :])

        # --- PE warmup: memset a small tile, then a burst of dummy matmuls
        # into a scratch PSUM bank so the HAM clock gate reaches K=8/8 and
        # stays there until the real matmul stream begins.
        nc.vector.memset(warm[:], 1.0)
        for _ in range(n_warm):
            nc.tensor.matmul(ps_warm[:], warm[:, 0:128], warm[:], start=True, stop=True)

        # --- dummy activation: forces ACT_TABLE_LOAD during the DMA window
        # (after the scalar-queue input DMA descriptor, before the relus).
        nc.vector.memset(dummy[:], 0.0)
        nc.scalar.activation(dummy2[:], dummy[:], ACT.Relu, bias=0.0)

        def as_groups(ap, ng):
            # (P, gc) dense tile viewed as (P, ng, fold) to match slot APs
            return bass.AP(
                tensor=ap.tensor,
                offset=ap.offset,
                ap=[ap.ap[0], [FOLD, ng], [1, FOLD]],
            )

        def a_flat(ci, gb):
            # contiguous (P, 128) view of group gb of the A tile (flat AP so
            # LDWEIGHTS sees a plain dense operand and FWL can trigger)
            ap = a_ts[ci][:]
            return bass.AP(
                tensor=ap.tensor,
                offset=ap.offset + gb * (M_A * FOLD),
                ap=[ap.ap[0], [1, M_A * FOLD]],
            )

        def c_slice(ci, gb):
            # (P, 17, fold) strided view of the slot-major C tile
            ap = c_ts[ci][:]
            gc = chunks[ci]
            return bass.AP(
                tensor=ap.tensor,
                offset=ap.offset + gb * FOLD,
                ap=[ap.ap[0], [gc, N_C], [1, FOLD]],
            )

        # ones columns for chunk0 up front on gpsimd; later chunks' ones are
        # interleaved after the previous chunk's gpsimd C-columns.
        def emit_ones(ci):
            ng = chunks[ci] // FOLD
            nc.gpsimd.memset(a_ts[ci][:, :ng, 0, :], 1.0)
            nc.gpsimd.memset(c_ts[ci][:, 0, :], 1.0)

        emit_ones(0)

        def dve_A(ci, a):
            ng = chunks[ci] // FOLD
            nc.vector.tensor_scalar(
                a_ts[ci][:, :ng, 1 + a, :], as_groups(xcs[ci][:], ng),
                float(4096 * (a + 1)) - 0.5, None, ALU.is_gt,
            )

        def dve_AND(ci):
            nc.vector.tensor_scalar(
                rrs[ci][:], xcs[ci][:], 0x0FFF, None, ALU.bitwise_and
            )

        # DVE queue head: rr for chunk0, a couple of A passes to cover the
        # later chunks' DMA latency, then rr for chunks 1..n so ScalarE's
        # relu stream never stalls mid-queue.
        dve_AND(0)
        dve_A(0, 0)
        dve_A(0, 1)
        dve_A(0, 2)
        for ci in range(1, len(chunks)):
            dve_AND(ci)

        mi = 0
        last_c = len(chunks) - 1
        n_mm0 = n_mm - chunks[last_c] // FOLD
        for ci, gc in enumerate(chunks):
            ng = gc // FOLD
            xc, rr, a_t, c_t = xcs[ci], rrs[ci], a_ts[ci], c_ts[ci]
            n_sc = sc_count[ci]

            # ScalarE C columns (need only rr)
            for lam in range(n_sc):
                nc.scalar.activation(
                    c_t[:, 1 + lam, :], rr[:], ACT.Relu, bias=bias_t[lam][:]
                )
            if ci + 1 < len(chunks):
                emit_ones(ci + 1)

            # Remaining A columns + DVE's share of C columns
            for a in range(3 if ci == 0 else 0, 15):
                dve_A(ci, a)
            for lam in range(n_sc, 16):
                nc.vector.tensor_scalar(
                    c_t[:, 1 + lam, :], rr[:], float(256 * lam), 0.0,
                    ALU.subtract, ALU.max,
                )

            ps = ps1 if ci == last_c else ps0
            for gb in range(ng):
                nc.tensor.matmul(
                    ps[:],
                    a_flat(ci, gb),
                    c_slice(ci, gb),
                    start=(mi == 0 or mi == n_mm0),
                    stop=(mi == n_mm0 - 1 or mi == n_mm - 1),
                )
                mi += 1

        # PSUM -> SBUF stages on DVE (DMA can't read PSUM); ps0's stage +
        # DMA overlap the last chunk's features/matmuls.
        stage = singles.tile([MR, 2 * MC], FP32, name="stage")
        nc.vector.tensor_copy(stage[:, 0:MC], ps0[:])
        nc.sync.dma_start(gacc_dram[:, 0:MC], stage[:, 0:MC])
        nc.vector.tensor_copy(stage[:, MC : 2 * MC], ps1[:])
        nc.sync.dma_start(gacc_dram[:, MC : 2 * MC], stage[:, MC : 2 * MC])

    orig = nc.to_json_bytes
    nc.to_json_bytes = lambda *a, **k: _split_multiwaits(orig(*a, **k))
    return nc


def _prep(images: np.ndarray) -> list:
    """(8,3,256,256) f32 -> per-core in_maps with uint16 fixed-point t16."""
    t16 = np.rint(np.asarray(images, np.float32) * 65280.0)
    t16 = np.clip(t16, 0.0, 65535.0).astype(np.uint16)
    return [{"x": t16[b].reshape(P, F).copy()} for b in range(t16.shape[0])]


def _gacc_to_hist(gacc: np.ndarray) -> np.ndarray:
    """(16*fold, 2*17*fold) two raw accumulator banks -> (256,) histogram."""
    acc = gacc.astype(np.float64)
    acc = acc[:, :N_C * FOLD] + acc[:, N_C * FOLD:]
    raw = np.zeros((M_A, N_C), np.float64)
    for gg in range(FOLD):
        raw += acc[gg::FOLD, gg::FOLD]
    # rows: [ones, S_0..S_14]; one-hot rows for h=0..15
    T = np.zeros((M_A + 1, N_C), np.float64)
    T[0:16] = raw
    O = T[0:16] - T[1:17]
    # columns: [count, R_0..R_15], R_lam = sum 256*relu(r - lam)
    C_a = O[:, 0]                      # (16,) counts per coarse bin
    R = O[:, 1:17] / 256.0             # (16, 16) relu sums
    B = np.zeros((M_A, 17), np.float64)  # clamp01 sums, lam=0..16
    B[:, 16] = 0.0
    B[:, 15] = R[:, 15]
    for lam in range(14, -1, -1):
        B[:, lam] = R[:, lam] - R[:, lam + 1]
    # tri sums: T[a,l] = B_{l-1} - B_l with B_{-1} = count; spill = B_15
    hist = np.zeros((M_A, 16), np.float64)
    hist[:, 0] = C_a - B[:, 0]
    for l in range(1, 16):
        hist[:, l] = B[:, l - 1] - B[:, l]
    hist[1:, 0] += B[:-1, 15]  # previous coarse bin's r=16 spill
    return hist.reshape(256).astype(np.float32)


_NC_CACHE = []


def kernel(images_batch: np.ndarray, bin_centers: np.ndarray) -> np.ndarray:
    images = np.asarray(images_batch, dtype=np.float32)
    assert images.shape == (N_CORES, 3, 256, 256), images.shape
    # bin_centers is linspace(0,1,256) by construction; the kernel math
    # hardcodes those bins (t = 255*x vs integer bin index).

    if not _NC_CACHE:
        _NC_CACHE.append(_build_program())
    nc = _NC_CACHE[0]

    in_maps = _prep(images)
    res = run_bass_kernel_spmd(nc, in_maps, core_ids=list(range(N_CORES)))
    return np.stack([_gacc_to_hist(res.results[b]["gacc"]) for b in range(N_CORES)])


if __name__ == "__main__":
    rng = np.random.default_rng(1)
    imgs = rng.random((8, 3, 256, 256), dtype=np.float32)
    bins = np.linspace(0.0, 1.0, 256, dtype=np.float32)
    out = kernel(images_batch=imgs, bin_centers=bins)
    t = imgs.reshape(8, -1).astype(np.float64) * 255.0
    j = np.arange(256)
    want = np.clip(1.0 - np.abs(t[:, :, None] - j[None, None, :]), 0, None).sum(1)
    rel = np.abs(out - want).max() / np.abs(want).max()
    print("self-test rel err:", rel)
    print("PASS" if rel < 2e-2 else "FAIL")


# revision 15
# speedup vs baseline: 1.0433x; 1.0433x over previous
"""Differentiable-histogram Trainium2 kernel (256 triangular bins), v2.

Contract: kernel(**inputs) takes the FULL inputs from setup_inputs()
(images_batch: (8,3,256,256) f32 in [0,1]; bin_centers: (256,) f32 =
linspace(0,1,256)) and returns the FULL (8,256) f32 histogram:

    hist[b, j] = sum_i relu(1 - |255*x_bi - j|)

Strategy (pure data parallel, one image per NeuronCore, 8 cores):
  Host uploads t16 = round(x*255*256) as uint16 (fixed point 8.8, halves
  the DMA bytes; quantization error <= 1/512 in t).  On device, with
  h = t16 >> 12 (coarse bin, 16 wide) and r16 = t16 & 0xFFF (= 256*r,
  r in [0,16)):
    A side (16 slots, matmul stationary): [ones, S_0..S_14],
      S_a = 1{t16 > 4096(a+1)-0.5} = 1{h > a}  -- DVE is_gt straight off
      t16, no float prologue needed.
    C side (17 slots, matmul moving): [ones, R_0..R_15],
      R_lam = relu(r16 - 256*lam) -- one pass per column, split across
      DVE (tensor_scalar sub/max), ScalarE (activation Relu) and GPSIMD
      (tensor_scalar) to use all three elementwise engines.
  One matmul per FOLD pixel-columns accumulates A^T C into a single PSUM
  bank; host extracts the fold diagonal and decodes with first/second
  differences (tri(r-l) = clamp01(r-l+1) - clamp01(r-l), and
  relu(r-lam) = sum_{mu>=lam} clamp01(r-mu)).

Perf structure: input DMA for chunk0 issued from the ScalarE queue
(ready ~3us before sync's), dummy Relu to pull ACT_TABLE_LOAD into the
DMA window, a burst of dummy matmuls to warm the PE HAM clock gate
before the real matmul stream, no SBUF buffer reuse (no WAR stalls),
and a small final chunk to keep the exposed matmul tail short.
"""

import json as _json
from contextlib import ExitStack

import numpy as np

import concourse.bass as bass
import concourse.tile as tile
from concourse import mybir
from concourse.bass_utils import run_bass_kernel_spmd

FP32 = mybir.dt.float32
BF16 = mybir.dt.bfloat16
U16 = mybir.dt.uint16
ALU = mybir.AluOpType
ACT = mybir.ActivationFunctionType

N_CORES = 8
P, F = 128, 1536  # per-core pixels: 3*256*256 = 196608 = 128*1536
FOLD = 8
M_A = 16  # lhsT slots: ones + S_0..S_14
N_C = 17  # rhs slots: ones + R_0..R_15
MR, MC = M_A * FOLD, N_C * FOLD

CHUNKS = (768, 640, 128)
# C-column engine assignment (lam -> engine). GPSIMD takes the low lams,
# ScalarE the middle, DVE the rest.
SC_COUNT = (8, 9, 9)  # per-chunk: how many C columns ScalarE computes
N_WARM_MM = 32   # dummy matmuls at startup to warm the PE HAM clock gate
RR_MODE = "and"  # "and" | "mod" : how r16 is computed from t16
RELU_BIAS_IMM = False  # immediate bias on ScalarE Relu (else bias tiles)


def _split_multiwaits(bir_bytes: bytes) -> bytes:
    """This container's walrus rejects any instruction carrying more than
    one sem wait. Split extras onto standalone EventSemaphore instructions;
    additionally drop the exit-drain's queue waits (NRT drains rings at
    exec end anyway)."""
    bir = _json.loads(bir_bytes)
    for fn in bir["functions"]:
        for blk in fn["blocks"]:
            is_end = str(blk.get("name", "")).endswith("_end")
            out = []
            for ins in blk["instructions"]:
                si = ins.get("sync_info")
                ow = (si or {}).get("on_wait") or []
                if is_end and ins.get("opcode") == "Drain" and len(ow) > 1:
                    si["on_wait"] = []
                elif len(ow) > 1:
                    for k, w in enumerate(ow[:-1]):
                        out.append(
                            {
                                "debug": ins.get("debug", 1),
                                "engine": ins["engine"],
                                "ins": [],
                                "name": f"{ins['name']}_w{k}",
                                "opcode": "EventSemaphore",
                                "outs": [],
                                "sync_info": {"on_update": [], "on_wait": [w]},
                            }
                        )
                    si["on_wait"] = [ow[-1]]
                out.append(ins)
            blk["instructions"] = out
    return _json.dumps(bir).encode()


def _build_program(chunks=CHUNKS, sc_count=SC_COUNT,
                   n_warm=N_WARM_MM, rr_mode=RR_MODE,
                   relu_bias_imm=RELU_BIAS_IMM):
    assert sum(chunks) == F
    n_mm = sum(gc // FOLD for gc in chunks)
    sc_lams = tuple(range(max(sc_count)))

    nc = bass.Bass("TRN2", target_bir_lowering=False)

    x_dram = nc.dram_tensor("x", [P, F], U16, kind="ExternalInput")
    gacc_dram = nc.dram_tensor("gacc", [MR, 2 * MC], FP32, kind="ExternalOutput")

    with tile.TileContext(nc) as tc, ExitStack() as ctx:
        singles = ctx.enter_context(tc.tile_pool(name="singles", bufs=1))
        psum_pool = ctx.enter_context(tc.tile_pool(name="psum", bufs=1, space="PSUM"))

        # Two accumulator banks: chunks 0..n-2 into ps0, last chunk into
        # ps1, so ps0's stage copy + output DMA overlap the final chunk.
        ps0 = psum_pool.tile([MR, MC], FP32, name="ps0")
        ps1 = psum_pool.tile([MR, MC], FP32, name="ps1")
        ps_warm = [psum_pool.tile([P, 512], FP32, name=f"ps_warm{i}") for i in range(2)]

        # Per-chunk tiles, no reuse anywhere (no WAR deps, no self-pacing).
        xcs, rrs, a_ts, c_ts = [], [], [], []
        for ci, gc in enumerate(chunks):
            ng = gc // FOLD
            xcs.append(singles.tile([P, gc], U16, name=f"x{ci}"))
            rrs.append(singles.tile([P, gc], U16, name=f"r{ci}"))
            a_ts.append(singles.tile([P, ng, M_A, FOLD], BF16, name=f"A{ci}"))
            c_ts.append(singles.tile([P, N_C, gc], BF16, name=f"C{ci}"))
        warm = singles.tile([P, 512], BF16, name="warm")
        dummy = singles.tile([P, 1], FP32, name="dummy")
        dummy2 = singles.tile([P, 1], FP32, name="dummy2")

        bias_t = {}
        for lam in sc_lams:
            bt = singles.tile([P, 1], FP32, name=f"bias{lam}")
            nc.vector.memset(bt[:], float(-256 * lam))
            bias_t[lam] = bt

        # --- input DMAs first, spread across the three DMA-capable queues
        # in the order their preambles clear: gpsimd (~7.2us), sync (~7.6),
        # scalar (~7.3, before its ACT_TABLE_LOAD).
        # chunk0 is the critical path: split its transfer across the scalar
        # and sync queues so it lands ~1.4us sooner; x1 follows on scalar,
        # x2 on sync.  gpsimd's queue stays free for the ones memsets.
        h0 = chunks[0] // 2
        c0, c1, c2 = chunks
        nc.scalar.dma_start(xcs[0][:, 0:h0], x_dram[:, 0:h0])
        nc.sync.dma_start(xcs[0][:, h0:], x_dram[:, h0:c0])
        nc.scalar.dma_start(xcs[1][:], x_dram[:, c0 : c0 + c1])
        nc.sync.dma_start(xcs[2][:], x_dram[:, c0 + c1 :])

        # --- PE warmup: memset a small tile, then a burst of dummy matmuls
        # into a scratch PSUM bank so the HAM clock gate reaches K=8/8 and
        # stays there until the real matmul stream begins.
        nc.vector.memset(warm[:], 1.0)
        for w in range(n_warm):
            nc.tensor.matmul(
                ps_warm[w % 2][:], warm[:, 0:128], warm[:],
                start=True, stop=True,
            )

        # --- dummy activation: forces ACT_TABLE_LOAD during the DMA window
        # (after the scalar-queue input DMA descriptor, before the relus).
        nc.vector.memset(dummy[:], 0.0)
        nc.scalar.activation(dummy2[:], dummy[:], ACT.Relu, bias=0.0)

        def as_groups(ap, ng):
            # (P, gc) dense tile viewed as (P, ng, fold) to match slot APs
            return bass.AP(
                tensor=ap.tensor,
                offset=ap.offset,
                ap=[ap.ap[0], [FOLD, ng], [1, FOLD]],
            )

        def a_flat(ci, gb):
            # contiguous (P, 128) view of group gb of the A tile (flat AP so
            # LDWEIGHTS sees a plain dense operand and FWL can trigger)
            ap = a_ts[ci][:]
            return bass.AP(
                tensor=ap.tensor,
                offset=ap.offset + gb * (M_A * FOLD),
                ap=[ap.ap[0], [1, M_A * FOLD]],
            )

        def c_slice(ci, gb):
            # (P, 17, fold) strided view of the slot-major C tile
            ap = c_ts[ci][:]
            gc = chunks[ci]
            return bass.AP(
                tensor=ap.tensor,
                offset=ap.offset + gb * FOLD,
                ap=[ap.ap[0], [gc, N_C], [1, FOLD]],
            )

        # ones columns for chunk0 up front on gpsimd; later chunks' ones are
        # interleaved after the previous chunk's gpsimd C-columns.
        def emit_ones(ci):
            ng = chunks[ci] // FOLD
            nc.gpsimd.memset(a_ts[ci][:, :ng, 0, :], 1.0)
            nc.gpsimd.memset(c_ts[ci][:, 0, :], 1.0)

        emit_ones(0)

        def dve_A(ci, a):
            ng = chunks[ci] // FOLD
            nc.vector.tensor_scalar(
                a_ts[ci][:, :ng, 1 + a, :], as_groups(xcs[ci][:], ng),
                float(4096 * (a + 1)) - 0.5, None, ALU.is_gt,
            )

        def dve_AND(ci):
            nc.vector.tensor_scalar(
                rrs[ci][:], xcs[ci][:], 0x0FFF, None, ALU.bitwise_and
            )

        # DVE queue head: rr for chunk0, a couple of A passes to cover the
        # later chunks' DMA latency, then rr for chunks 1..n so ScalarE's
        # relu stream never stalls mid-queue.
        dve_AND(0)
        dve_A(0, 0)
        dve_A(0, 1)
        dve_A(0, 2)
        for ci in range(1, len(chunks)):
            dve_AND(ci)

        mi = 0
        last_c = len(chunks) - 1
        n_mm0 = n_mm - chunks[last_c] // FOLD
        for ci, gc in enumerate(chunks):
            ng = gc // FOLD
            xc, rr, a_t, c_t = xcs[ci], rrs[ci], a_ts[ci], c_ts[ci]
            n_sc = sc_count[ci]

            # ScalarE C columns (need only rr)
            for lam in range(n_sc):
                nc.scalar.activation(
                    c_t[:, 1 + lam, :], rr[:], ACT.Relu, bias=bias_t[lam][:]
                )
            if ci + 1 < len(chunks):
                emit_ones(ci + 1)

            # Remaining A columns + DVE's share of C columns
            for a in range(3 if ci == 0 else 0, 15):
                dve_A(ci, a)
            for lam in range(n_sc, 16):
                nc.vector.tensor_scalar(
                    c_t[:, 1 + lam, :], rr[:], float(256 * lam), 0.0,
                    ALU.subtract, ALU.max,
                )

            ps = ps1 if ci == last_c else ps0
            for gb in range(ng):
                nc.tensor.matmul(
                    ps[:],
                    a_flat(ci, gb),
                    c_slice(ci, gb),
                    start=(mi == 0 or mi == n_mm0),
                    stop=(mi == n_mm0 - 1 or mi == n_mm - 1),
                )
                mi += 1

        # PSUM -> SBUF stages on DVE (DMA can't read PSUM); ps0's stage +
        # DMA overlap the last chunk's features/matmuls.
        stage = singles.tile([MR, 2 * MC], FP32, name="stage")
        nc.vector.tensor_copy(stage[:, 0:MC], ps0[:])
        nc.sync.dma_start(gacc_dram[:, 0:MC], stage[:, 0:MC])
        nc.vector.tensor_copy(stage[:, MC : 2 * MC], ps1[:])
        nc.sync.dma_start(gacc_dram[:, MC : 2 * MC], stage[:, MC : 2 * MC])

    orig = nc.to_json_bytes
    nc.to_json_bytes = lambda *a, **k: _split_multiwaits(orig(*a, **k))
    return nc


def _prep(images: np.ndarray) -> list:
    """(8,3,256,256) f32 -> per-core in_maps with uint16 fixed-point t16."""
    t16 = np.rint(np.asarray(images, np.float32) * 65280.0)
    t16 = np.clip(t16, 0.0, 65535.0).astype(np.uint16)
    return [{"x": t16[b].reshape(P, F).copy()} for b in range(t16.shape[0])]


def _gacc_to_hist(gacc: np.ndarray) -> np.ndarray:
    """(16*fold, 2*17*fold) two raw accumulator banks -> (256,) histogram."""
    acc = gacc.astype(np.float64)
    acc = acc[:, :N_C * FOLD] + acc[:, N_C * FOLD:]
    raw = np.zeros((M_A, N_C), np.float64)
    for gg in range(FOLD):
        raw += acc[gg::FOLD, gg::FOLD]
    # rows: [ones, S_0..S_14]; one-hot rows for h=0..15
    T = np.zeros((M_A + 1, N_C), np.float64)
    T[0:16] = raw
    O = T[0:16] - T[1:17]
    # columns: [count, R_0..R_15], R_lam = sum 256*relu(r - lam)
    C_a = O[:, 0]                      # (16,) counts per coarse bin
    R = O[:, 1:17] / 256.0             # (16, 16) relu sums
    B = np.zeros((M_A, 17), np.float64)  # clamp01 sums, lam=0..16
    B[:, 16] = 0.0
    B[:, 15] = R[:, 15]
    for lam in range(14, -1, -1):
        B[:, lam] = R[:, lam] - R[:, lam + 1]
    # tri sums: T[a,l] = B_{l-1} - B_l with B_{-1} = count; spill = B_15
    hist = np.zeros((M_A, 16), np.float64)
    hist[:, 0] = C_a - B[:, 0]
    for l in range(1, 16):
        hist[:, l] = B[:, l - 1] - B[:, l]
    hist[1:, 0] += B[:-1, 15]  # previous coarse bin's r=16 spill
    return hist.reshape(256).astype(np.float32)


_NC_CACHE = []


def kernel(images_batch: np.ndarray, bin_centers: np.ndarray) -> np.ndarray:
    images = np.asarray(images_batch, dtype=np.float32)
    assert images.shape == (N_CORES, 3, 256, 256), images.shape
    # bin_centers is linspace(0,1,256) by construction; the kernel math
    # hardcodes those bins (t = 255*x vs integer bin index).

    if not _NC_CACHE:
        _NC_CACHE.append(_build_program())
    nc = _NC_CACHE[0]

    in_maps = _prep(images)
    res = run_bass_kernel_spmd(nc, in_maps, core_ids=list(range(N_CORES)))
    return np.stack([_gacc_to_hist(res.results[b]["gacc"]) for b in range(N_CORES)])


if __name__ == "__main__":
    rng = np.random.default_rng(1)
    imgs = rng.random((8, 3, 256, 256), dtype=np.float32)
    bins = np.linspace(0.0, 1.0, 256, dtype=np.float32)
    out = kernel(images_batch=imgs, bin_centers=bins)
    t = imgs.reshape(8, -1).astype(np.float64) * 255.0
    j = np.arange(256)
    want = np.clip(1.0 - np.abs(t[:, :, None] - j[None, None, :]), 0, None).sum(1)
    rel = np.abs(out - want).max() / np.abs(want).max()
    print("self-test rel err:", rel)
    print("PASS" if rel < 2e-2 else "FAIL")
